# revision 25
# baseline (speedup 1.0000x reference)
"""3-layer GAT on 8 TRN2 NeuronCores.

Sharding: nodes/edges partitioned by destination node across 8 cores
(1250 rows each); weights replicated. Per layer: each core projects its own
rows (h @ W_ext, where W_ext also produces the per-node attention terms
e_src/e_dst), AllGather of the projected table, then per-core edge
processing: dma_gather of source rows, attention softmax (normalization
folded to after aggregation), and segment-sum aggregation via one-hot
matmuls into PSUM. e_dst is expanded from own rows via a transposed
one-hot matmul (no second gather). The next layer's dense projection is
interleaved into the edge phase chunk-by-chunk. Final global mean pool +
FC + log_softmax with one small AllReduce.

Self-contained: hardcodes all shapes for the nn_AdjustedGATModel problem
(N=10000, E=160000, F_IN=512, HID=1024, HEADS=4, L=3, G=16, NC=10).
"""
import sys

sys.path.insert(0, "/opt/trn_rl_repo")

import numpy as np
import ml_dtypes

import concourse.bacc as bacc
import concourse.mybir as mybir
import concourse.tile as tile
from concourse.bass_utils import run_bass_kernel_spmd

dt = mybir.dt
BF16 = ml_dtypes.bfloat16

NCORES = 8
N, E, F_IN, DIM, HEADS, L, G, NCLS = 10000, 160000, 512, 256, 4, 3, 16, 10
HID = HEADS * DIM                # 1024
ROWS = N // NCORES               # 1250
ROWS_PAD = 1280
NCHUNK = 10                      # dst chunks of 128 rows
T_TILES = 18                     # edge tiles (of 128) per chunk
ESLOT = T_TILES * 128            # 2432 edge slots per chunk
HALves = ((0, 9), (9, 9))        # (tile0, ntiles) per half-chunk gather
HT0 = 9
TBL = 1152                       # int16 cols per table row (bf16 h + f32 es/ed)
IDXW = ESLOT // 16               # 152 idx cols per chunk
TBL_ROWS = NCORES * ROWS_PAD     # 10240
KCH = (F_IN // 128, HID // 128, HID // 128)   # k-chunks per layer: 4,8,8
PAD_DST = 255.0                  # sentinel dst id for pad edge slots
AG_LO = (0, 384, 768, 1152)      # row starts of the 4 AllGather splits
AG_SZ = (384, 384, 384, 128)     # rows per split
AG_BASE = (0, 3072, 6144, 9216)  # table row base of each split
AG_END_CH = (2, 5, 8, 9)         # split fires after this dense chunk

_NC = None


def ag_part(nc, agin_t, table_t, k):
    """AllGather split k of the projected rows; early splits overlap the
    tail of the producing phase. Table rows: rank-major within a split."""
    lo, sz, base = AG_LO[k], AG_SZ[k], AG_BASE[k]
    nc.gpsimd.collective_compute(
        "AllGather", mybir.AluOpType.bypass,
        replica_groups=[list(range(NCORES))],
        ins=[agin_t[lo:lo + sz, :]],
        outs=[table_t[base:base + NCORES * sz, :]])


def build():
    nc = bacc.Bacc("TRN2", num_devices=NCORES, target_bir_lowering=False)
    P = nc.declare_dram_parameter

    xT = P("xT", [F_IN, ROWS_PAD], dt.bfloat16, isOutput=False)
    w = [P(f"w{l}", [KCH[l] * 128, HID + 8], dt.bfloat16, isOutput=False)
         for l in range(L)]
    biasb = P("biasb", [128, L * HID], dt.float32, isOutput=False)
    fcw = P("fcw", [128, 8 * NCLS], dt.float32, isOutput=False)
    fcb = P("fcb", [G, NCLS], dt.float32, isOutput=False)
    crec = P("crec", [G, 1], dt.float32, isOutput=False)
    srcidx = P("srcidx", [128, NCHUNK * IDXW], dt.int16, isOutput=False)
    dstloc = P("dstloc", [128, NCHUNK * T_TILES], dt.bfloat16, isOutput=False)
    dstrow = P("dstrow", [128, NCHUNK * ESLOT], dt.bfloat16, isOutput=False)
    bo = P("bo", [128, NCHUNK * 17], dt.bfloat16, isOutput=False)
    fiota = P("fiota", [128, ESLOT], dt.bfloat16, isOutput=False)
    piota = P("piota", [128, 1], dt.float32, isOutput=False)
    out = P("out", [G, NCLS], dt.float32, isOutput=True)

    agin = [nc.dram_tensor(f"agin{l}", [ROWS_PAD, TBL], dt.int16)
            for l in range(L)]
    table = [nc.dram_tensor(f"table{l}", [TBL_ROWS, TBL], dt.int16,
                            addr_space="Shared") for l in range(L)]
    hbuf = [nc.dram_tensor(f"hbuf{l}", [ROWS_PAD, HID], dt.bfloat16)
            for l in range(L - 1)]
    arin = nc.dram_tensor("arin", [128, 136], dt.float32)
    arout = nc.dram_tensor("arout", [128, 136], dt.float32,
                           addr_space="Shared")

    with tile.TileContext(nc) as tc:
        import contextlib
        with contextlib.ExitStack() as ctx:
            const = ctx.enter_context(tc.tile_pool(name="const", bufs=1))
            wpool = ctx.enter_context(tc.tile_pool(name="wpool", bufs=1))
            hTp = ctx.enter_context(tc.tile_pool(name="hTp", bufs=1))
            gp = ctx.enter_context(tc.tile_pool(name="gp", bufs=4))
            drp = ctx.enter_context(tc.tile_pool(name="drp", bufs=3))
            s2p = ctx.enter_context(tc.tile_pool(name="s2p", bufs=3))
            stp = ctx.enter_context(tc.tile_pool(name="stp", bufs=3))
            ep = ctx.enter_context(tc.tile_pool(name="ep", bufs=3))
            hout = ctx.enter_context(tc.tile_pool(name="hout", bufs=2))
            dns = ctx.enter_context(tc.tile_pool(name="dns", bufs=2))
            edp = ctx.enter_context(tc.tile_pool(name="edp", bufs=2))
            bip = ctx.enter_context(tc.tile_pool(name="bip", bufs=1))
            psA = ctx.enter_context(tc.tile_pool(name="psA", bufs=2,
                                                 space="PSUM"))
            psX = ctx.enter_context(tc.tile_pool(name="psX", bufs=1,
                                                 space="PSUM"))
            psE = ctx.enter_context(tc.tile_pool(name="psE", bufs=2,
                                                 space="PSUM"))
            psD = ctx.enter_context(tc.tile_pool(name="psD", bufs=1,
                                                 space="PSUM"))

            # ---- constants to SBUF ----
            fio = const.tile([128, ESLOT], dt.bfloat16)
            nc.sync.dma_start(fio[:], fiota[:])
            pio = const.tile([128, 1], dt.float32)
            nc.sync.dma_start(pio[:], piota[:])
            dl_sb = const.tile([128, NCHUNK * T_TILES], dt.bfloat16)
            nc.sync.dma_start(dl_sb[:], dstloc[:])
            si_sb = const.tile([128, NCHUNK * IDXW], dt.int16)
            nc.sync.dma_start(si_sb[:], srcidx[:])
            bo_sb = const.tile([128, NCHUNK * 17], dt.bfloat16)
            nc.sync.dma_start(bo_sb[:], bo[:])
            fcw_sb = const.tile([128, 8 * NCLS], dt.float32)
            nc.sync.dma_start(fcw_sb[:], fcw[:])
            fcb_sb = const.tile([G, NCLS], dt.float32)
            nc.sync.dma_start(fcb_sb[:], fcb[:])
            crec_sb = const.tile([G, 1], dt.float32)
            nc.sync.dma_start(crec_sb[:], crec[:])
            poolacc = const.tile([128, 8 * 17], dt.float32)
            nc.vector.memset(poolacc[:], 0.0)

            w_sbs = [None] * L

            def load_w(l):
                tagw = "wA" if l != 1 else "wB"
                w_sb = wpool.tile([128, 8, HID + 8], dt.bfloat16,
                                  tag=tagw, name=f"w{l}")
                for k in range(KCH[l]):
                    nc.sync.dma_start(w_sb[:, k, :],
                                      w[l][k * 128:(k + 1) * 128, :])
                w_sbs[l] = w_sb

            def dense_chunk(l, m, hT_sb, edown):
                K = KCH[l]
                w_sb = w_sbs[l]
                y1 = psD.tile([128, 512], dt.float32, space="PSUM",
                              tag="y1", name=f"y1_{l}_{m}")
                hb = dns.tile([128, HID], dt.bfloat16, tag="hb")
                for half in range(2):
                    for k in range(K):
                        nc.tensor.matmul(
                            y1[:], lhsT=hT_sb[:, k, m * 128:(m + 1) * 128],
                            rhs=w_sb[:, k, half * 512:(half + 1) * 512],
                            start=(k == 0), stop=(k == K - 1))
                    nc.vector.tensor_copy(
                        hb[:, half * 512:(half + 1) * 512], y1[:])
                for k in range(K):
                    nc.tensor.matmul(
                        y1[:, 0:8], lhsT=hT_sb[:, k, m * 128:(m + 1) * 128],
                        rhs=w_sb[:, k, 1024:1032],
                        start=(k == 0), stop=(k == K - 1))
                ee = dns.tile([128, 64], dt.float32, tag="ee")
                nc.vector.tensor_copy(ee[:, 0:8], y1[:, 0:8])
                nc.vector.memset(ee[:, 8:64], 0.0)
                nc.vector.tensor_copy(edown[:, m, :], y1[:, 4:8])
                r0 = m * 128
                nc.sync.dma_start(
                    agin[l][r0:r0 + 128, 0:HID].bitcast(dt.bfloat16), hb[:])
                nc.sync.dma_start(
                    agin[l][r0:r0 + 128, HID:TBL].bitcast(dt.float32), ee[:])

            # ---- layer 0 dense from xT ----
            load_w(0)
            xT_sb = hTp.tile([128, 8, ROWS_PAD], dt.bfloat16, tag="hT",
                             name="xT_sb")
            for k in range(KCH[0]):
                nc.sync.dma_start(xT_sb[:, k, :],
                                  xT[k * 128:(k + 1) * 128, :])
            edowns = [None] * L
            edowns[0] = edp.tile([128, NCHUNK, 4], dt.bfloat16,
                                 tag="edown", name="edown0")
            for m in range(NCHUNK):
                dense_chunk(0, m, xT_sb, edowns[0])
                if m in AG_END_CH:
                    ag_part(nc, agin[0], table[0], AG_END_CH.index(m))
            load_w(1)

            hT_next = None
            for l in range(L):
                if l < L - 1:
                    hT_next = hTp.tile([128, 8, ROWS_PAD], dt.bfloat16,
                                       tag="hT", name=f"hT{l + 1}")
                    edowns[l + 1] = edp.tile([128, NCHUNK, 4], dt.bfloat16,
                                             tag="edown",
                                             name=f"edown{l + 1}")
                if l == 1:
                    load_w(2)
                bias_l = bip.tile([128, HID], dt.float32, tag="bias",
                                  name=f"bias{l}")
                nc.sync.dma_start(bias_l[:], biasb[:, l * HID:(l + 1) * HID])
                edown = edowns[l]

                for ch in range(NCHUNK):
                    a1 = psA.tile([128, 512], dt.float32, space="PSUM",
                                  tag="x1", name=f"a1_{l}_{ch}")
                    a2 = psA.tile([128, 512], dt.float32, space="PSUM",
                                  tag="x2", name=f"a2_{l}_{ch}")
                    a3 = psX.tile([128, 8], dt.float32, space="PSUM",
                                  tag="x3", name=f"a3_{l}_{ch}")

                    dr = drp.tile([128, ESLOT], dt.bfloat16, tag="dr")
                    nc.scalar.dma_start(
                        dr[:], dstrow[:, ch * ESLOT:(ch + 1) * ESLOT])
                    # one-hot (edge -> own-dst-row), and its transpose-ish
                    # twin (own-dst-row -> edge), one op per chunk each
                    st = stp.tile([128, ESLOT], dt.bfloat16, tag="st")
                    nc.vector.tensor_tensor(
                        st[:].rearrange("p (t d) -> p t d", d=128),
                        fio[:].rearrange("p (t d) -> p t d", d=128),
                        dl_sb[:, ch * T_TILES:(ch + 1) * T_TILES]
                        .to_broadcast([128, T_TILES, 128]),
                        op=mybir.AluOpType.is_equal)
                    s2 = s2p.tile([128, ESLOT], dt.bfloat16, tag="s2")
                    nc.vector.tensor_scalar(
                        s2[:], dr[:], pio[:, 0:1], None,
                        mybir.AluOpType.is_equal)
                    for hf, (t0, T) in enumerate(HALves):
                        nidx = T * 128
                        c0 = ch * IDXW + t0 * 8
                        G_t = gp.tile([128, HT0, TBL], dt.int16, tag="G")
                        nc.gpsimd.dma_gather(
                            G_t[:, 0:T, :], table[l][:],
                            si_sb[:, c0:c0 + nidx // 16], nidx, nidx,
                            TBL, single_packet=False)
                        edx = psE.tile([128, HT0 * 4], dt.float32,
                                       space="PSUM", tag="edx",
                                       name=f"edx_{l}_{ch}_{hf}")
                        for t in range(T):
                            nc.tensor.matmul(
                                edx[:, t * 4:(t + 1) * 4],
                                lhsT=s2[:, (t0 + t) * 128:
                                        (t0 + t + 1) * 128],
                                rhs=edown[:, ch, :], start=True, stop=True)
                        es = G_t[:, 0:T, 1024:1032].bitcast(dt.float32)
                        e1 = ep.tile([128, HT0, 4], dt.float32, tag="e1")
                        nc.vector.tensor_tensor(
                            e1[:, 0:T, :], es,
                            edx[:, 0:T * 4].rearrange(
                                "p (t f) -> p t f", f=4),
                            op=mybir.AluOpType.add)
                        e2 = ep.tile([128, HT0, 4], dt.float32, tag="e2")
                        nc.vector.tensor_scalar_mul(e2[:, 0:T, :],
                                                    e1[:, 0:T, :], 0.2)
                        nc.vector.tensor_tensor(e1[:, 0:T, :],
                                                e1[:, 0:T, :],
                                                e2[:, 0:T, :],
                                                op=mybir.AluOpType.max)
                        pbf = ep.tile([128, HT0, 4], dt.bfloat16, tag="pbf")
                        nc.scalar.activation(pbf[:, 0:T, :], e1[:, 0:T, :],
                                             mybir.ActivationFunctionType.Exp)
                        # scale gathered h rows by p ((f,h)-interleaved
                        # layout keeps the innermost stream step-1 bf16)
                        for g0, gn in ((0, 5), (5, T - 5)):
                            gh = G_t[:, g0:g0 + gn, 0:HID] \
                                .bitcast(dt.bfloat16) \
                                .rearrange("p t (o h) -> p t o h", h=4)
                            pb = pbf[:, g0:g0 + gn, :] \
                                .to_broadcast([128, gn, 4, DIM]) \
                                .rearrange("p t h o -> p t o h")
                            nc.vector.tensor_tensor(
                                gh, gh, pb, op=mybir.AluOpType.mult)
                        for t in range(T):
                            tg = t0 + t
                            lhs = st[:, tg * 128:(tg + 1) * 128]
                            first = (tg == 0)
                            last = (tg == T_TILES - 1)
                            gb = G_t[:, t, :].bitcast(dt.bfloat16)
                            nc.tensor.matmul(a1[:], lhsT=lhs,
                                             rhs=gb[:, 0:512],
                                             start=first, stop=last)
                            nc.tensor.matmul(a2[:], lhsT=lhs,
                                             rhs=gb[:, 512:1024],
                                             start=first, stop=last)
                            nc.tensor.matmul(a3[:, 0:4], lhsT=lhs,
                                             rhs=pbf[:, t, :],
                                             start=first, stop=last)
                    # normalize + bias + relu
                    den = hout.tile([128, 4], dt.float32, tag="den")
                    nc.vector.tensor_scalar_max(den[:], a3[:, 0:4], 1e-30)
                    rec = hout.tile([128, 4], dt.float32, tag="rec")
                    nc.vector.reciprocal(rec[:], den[:])
                    hn = hout.tile([128, HID], dt.float32, tag="hn")
                    recb = rec[:].to_broadcast([128, 4, 128]) \
                        .rearrange("p h o -> p o h")
                    for half, ap_ in ((0, a1), (1, a2)):
                        nc.vector.tensor_tensor(
                            hn[:, half * 512:(half + 1) * 512]
                            .rearrange("p (o h) -> p o h", h=4),
                            ap_[:].rearrange("p (o h) -> p o h", h=4),
                            recb, op=mybir.AluOpType.mult)
                    nc.vector.tensor_tensor(
                        hn[:], hn[:], bias_l[:],
                        op=mybir.AluOpType.add)
                    hrelu = hout.tile([128, HID], dt.bfloat16, tag="hrelu")
                    nc.scalar.activation(hrelu[:], hn[:],
                                         mybir.ActivationFunctionType.Relu)
                    if l < L - 1:
                        nc.sync.dma_start(
                            hbuf[l][ch * 128:(ch + 1) * 128, :], hrelu[:])
                        for k in range(8):
                            nc.sync.dma_start(
                                hT_next[:, k, ch * 128:(ch + 1) * 128],
                                hbuf[l][ch * 128:(ch + 1) * 128,
                                        k * 128:(k + 1) * 128],
                                transpose=True)
                        dense_chunk(l + 1, ch, hT_next, edowns[l + 1])
                        if ch in AG_END_CH[:3]:
                            ag_part(nc, agin[l + 1], table[l + 1],
                                    AG_END_CH.index(ch))
                    else:
                        boch = bo_sb[:, ch * 17:(ch + 1) * 17]
                        pl = psX.tile([128, 8 * 17], dt.float32,
                                      space="PSUM", tag="x3",
                                      name=f"plc{ch}")
                        for kf in range(8):
                            nc.tensor.matmul(
                                pl[:, kf * 17:(kf + 1) * 17],
                                lhsT=hrelu[:, kf * 128:(kf + 1) * 128],
                                rhs=boch, start=True, stop=True)
                        nc.vector.tensor_tensor(
                            poolacc[:], poolacc[:], pl[:],
                            op=mybir.AluOpType.add)
                if l < L - 1:
                    ag_part(nc, agin[l + 1], table[l + 1], 3)

            # ---- pooling epilogue: AllReduce + FC + log_softmax ----
            nc.sync.dma_start(arin[:], poolacc[:])
            nc.gpsimd.collective_compute(
                "AllReduce", mybir.AluOpType.add,
                replica_groups=[list(range(NCORES))],
                ins=[arin[:]], outs=[arout[:]])
            pool2 = hout.tile([128, 136], dt.float32, tag="pool2")
            nc.sync.dma_start(pool2[:], arout[:])
            lgp = psX.tile([G, NCLS], dt.float32, space="PSUM",
                           tag="x3", name="lgp")
            for kf in range(8):
                nc.tensor.matmul(lgp[:], lhsT=pool2[:, kf * 17:kf * 17 + G],
                                 rhs=fcw_sb[:, kf * NCLS:(kf + 1) * NCLS],
                                 start=(kf == 0), stop=(kf == 7))
            lg = hout.tile([G, NCLS], dt.float32, tag="lg")
            nc.vector.tensor_scalar_mul(lg[:], lgp[:], crec_sb[:, 0:1])
            nc.vector.tensor_tensor(lg[:], lg[:], fcb_sb[:],
                                    op=mybir.AluOpType.add)
            mx = hout.tile([G, 1], dt.float32, tag="mx")
            nc.vector.reduce_max(mx[:], lg[:], axis=mybir.AxisListType.X)
            xs = hout.tile([G, NCLS], dt.float32, tag="xs")
            nc.vector.tensor_scalar(xs[:], lg[:], mx[:, 0:1], None,
                                    mybir.AluOpType.subtract)
            ex = hout.tile([G, NCLS], dt.float32, tag="ex")
            nc.scalar.activation(ex[:], xs[:],
                                 mybir.ActivationFunctionType.Exp)
            sm = hout.tile([G, 1], dt.float32, tag="sm")
            nc.vector.reduce_sum(sm[:], ex[:], axis=mybir.AxisListType.X)
            lnv = hout.tile([G, 1], dt.float32, tag="lnv")
            nc.scalar.activation(lnv[:], sm[:],
                                 mybir.ActivationFunctionType.Ln)
            res = hout.tile([G, NCLS], dt.float32, tag="res")
            nc.vector.tensor_scalar(res[:], xs[:], lnv[:, 0:1], None,
                                    mybir.AluOpType.subtract)
            nc.sync.dma_start(out[:], res[:])

    nc.compile()
    return nc


def _padrow(n):
    n = np.asarray(n)
    r = n // ROWS
    loc = n % ROWS
    res = np.zeros_like(n)
    for k in range(4):
        lo, sz, base = AG_LO[k], AG_SZ[k], AG_BASE[k]
        m = (loc >= lo) & (loc < lo + sz)
        res = np.where(m, base + sz * r + (loc - lo), res)
    return res


_SIG = (np.arange(HID) % HEADS) * DIM + np.arange(HID) // HEADS  # new<-old


def _wrap16(idx):
    """[n] int16 -> [128, n//16] wrapped + replicated for the 8 Q7 cores."""
    n = len(idx)
    w = np.zeros((16, n // 16), np.int16)
    w[np.arange(n) % 16, np.arange(n) // 16] = idx
    return np.tile(w, (8, 1))


def prep(x, edge_index, batch, W0, W1, W2, a_src, a_dst, bias, fc_w, fc_b):
    """Host-side sharding/prep. Returns in_maps (list of dicts per core)."""
    x = np.asarray(x, np.float32)
    edge_index = np.asarray(edge_index)
    batch = np.asarray(batch)
    Ws = [np.asarray(W0, np.float32), np.asarray(W1, np.float32),
          np.asarray(W2, np.float32)]
    a_src = np.asarray(a_src, np.float32)
    a_dst = np.asarray(a_dst, np.float32)
    bias = np.asarray(bias, np.float32)
    fc_w = np.asarray(fc_w, np.float32)
    fc_b = np.asarray(fc_b, np.float32)

    src = np.concatenate([edge_index[0], np.arange(N, dtype=np.int64)])
    dst = np.concatenate([edge_index[1], np.arange(N, dtype=np.int64)])
    order = np.argsort(dst, kind="stable")
    src, dst = src[order], dst[order]

    wext = []
    for l in range(L):
        As = np.zeros((HID, HEADS), np.float32)
        Ad = np.zeros((HID, HEADS), np.float32)
        for h in range(HEADS):
            As[h * DIM:(h + 1) * DIM, h] = a_src[l, h]
            Ad[h * DIM:(h + 1) * DIM, h] = a_dst[l, h]
        we = np.concatenate([Ws[l], Ws[l] @ As, Ws[l] @ Ad], axis=1)
        we[:, 0:HID] = we[:, _SIG]          # interleave output cols (f,h)
        if l > 0:
            we = we[_SIG, :]                # permute input rows to match
        wext.append(np.ascontiguousarray(we).astype(BF16))

    bias_p = bias[:, _SIG]
    biasb = np.broadcast_to(bias_p.reshape(1, L * HID),
                            (128, L * HID)).copy()
    fcw = fc_w[_SIG, :]
    fcw = fcw.reshape(8, 128, NCLS).transpose(1, 0, 2).reshape(128, 8 * NCLS)
    fcw = np.ascontiguousarray(fcw, np.float32)
    fcb = np.tile(fc_b.reshape(1, NCLS), (G, 1)).astype(np.float32)
    cnts = np.bincount(np.asarray(batch, np.int64),
                       minlength=G).astype(np.float32)
    crec_h = (1.0 / np.maximum(cnts, 1.0)).reshape(G, 1).astype(np.float32)
    fiota = np.broadcast_to(
        np.tile(np.arange(128, dtype=np.float32), T_TILES).astype(BF16),
        (128, ESLOT)).copy()
    piota = np.arange(128, dtype=np.float32).reshape(128, 1)

    in_maps = []
    for c in range(NCORES):
        lo, hi = c * ROWS, (c + 1) * ROWS
        m = (dst >= lo) & (dst < hi)
        s_c, d_c = src[m], dst[m] - lo
        srcidx = np.zeros((128, NCHUNK * IDXW), np.int16)
        dstloc = np.zeros((128, NCHUNK * T_TILES), BF16)
        dstrow = np.zeros((NCHUNK, ESLOT), np.float32)
        for ch in range(NCHUNK):
            mm = (d_c >= ch * 128) & (d_c < (ch + 1) * 128)
            s_e, d_e = s_c[mm], d_c[mm] - ch * 128
            cnt = len(s_e)
            assert cnt <= ESLOT, f"core {c} chunk {ch}: {cnt} > {ESLOT}"
            sfull = np.zeros(ESLOT, np.int64)
            sfull[:cnt] = _padrow(s_e)
            for t0, T in HALves:
                cc = ch * IDXW + t0 * 8
                srcidx[:, cc:cc + T * 8] = _wrap16(
                    sfull[t0 * 128:(t0 + T) * 128].astype(np.int16))
            dl = np.full(ESLOT, PAD_DST, np.float32)
            dl[:cnt] = d_e
            dstloc[:, ch * T_TILES:(ch + 1) * T_TILES] = \
                dl.reshape(T_TILES, 128).T.astype(BF16)
            dstrow[ch] = dl
        dstrow_b = np.broadcast_to(
            dstrow.reshape(1, NCHUNK * ESLOT).astype(BF16),
            (128, NCHUNK * ESLOT)).copy()
        xT = np.zeros((F_IN, ROWS_PAD), BF16)
        xT[:, :ROWS] = x[lo:hi].T.astype(BF16)
        bo = np.zeros((ROWS_PAD, 17), np.float32)
        bo[np.arange(ROWS), batch[lo:hi]] = 1.0
        bo[:ROWS, 16] = 1.0
        bo_t = np.zeros((128, NCHUNK * 17), BF16)
        for ch in range(NCHUNK):
            bo_t[:, ch * 17:(ch + 1) * 17] = \
                bo[ch * 128:(ch + 1) * 128].astype(BF16)
        in_maps.append({
            "xT": xT, "w0": wext[0], "w1": wext[1], "w2": wext[2],
            "biasb": biasb, "fcw": fcw, "fcb": fcb, "crec": crec_h,
            "srcidx": srcidx, "dstloc": dstloc, "dstrow": dstrow_b,
            "bo": bo_t, "fiota": fiota, "piota": piota,
        })
    return in_maps


def run(inputs, trace=False, **kw):
    """Returns BassKernelResults (results + exec_time_ns when trace=True)."""
    global _NC
    if _NC is None:
        _NC = build()
    in_maps = prep(**inputs)
    return run_bass_kernel_spmd(_NC, in_maps, core_ids=list(range(NCORES)),
                                trace=trace, **kw)


def kernel(**inputs) -> np.ndarray:
    r = run(inputs)
    return np.asarray(r.results[0]["out"], np.float32)


# revision 26
# speedup vs baseline: 1.0566x; 1.0566x over previous
"""3-layer GAT on 8 TRN2 NeuronCores.

Sharding: nodes/edges partitioned by destination node across 8 cores
(1250 rows each); weights replicated. Per layer: each core projects its own
rows (h @ W_ext, where W_ext also produces the per-node attention terms
e_src/e_dst), AllGather of the projected table, then per-core edge
processing: dma_gather of source rows, attention softmax (normalization
folded to after aggregation), and segment-sum aggregation via one-hot
matmuls into PSUM. e_dst is expanded from own rows via a transposed
one-hot matmul (no second gather). The next layer's dense projection is
interleaved into the edge phase chunk-by-chunk. Final global mean pool +
FC + log_softmax with one small AllReduce.

Self-contained: hardcodes all shapes for the nn_AdjustedGATModel problem
(N=10000, E=160000, F_IN=512, HID=1024, HEADS=4, L=3, G=16, NC=10).
"""
import sys

sys.path.insert(0, "/opt/trn_rl_repo")

import numpy as np
import ml_dtypes

import concourse.bacc as bacc
import concourse.mybir as mybir
import concourse.tile as tile
from concourse.bass_utils import run_bass_kernel_spmd

dt = mybir.dt
BF16 = ml_dtypes.bfloat16

NCORES = 8
N, E, F_IN, DIM, HEADS, L, G, NCLS = 10000, 160000, 512, 256, 4, 3, 16, 10
HID = HEADS * DIM                # 1024
ROWS = N // NCORES               # 1250
ROWS_PAD = 1280
NCHUNK = 10                      # dst chunks of 128 rows
T_TILES = 18                     # edge tiles (of 128) per chunk
ESLOT = T_TILES * 128            # 2432 edge slots per chunk
HALves = ((0, 9), (9, 9))        # (tile0, ntiles) per half-chunk gather
HT0 = 9
TBL = 1152                       # int16 cols per table row (bf16 h + f32 es/ed)
IDXW = ESLOT // 16               # 152 idx cols per chunk
TBL_ROWS = NCORES * ROWS_PAD     # 10240
KCH = (F_IN // 128, HID // 128, HID // 128)   # k-chunks per layer: 4,8,8
PAD_DST = 255.0                  # sentinel dst id for pad edge slots
AG_LO = (0, 384, 768, 1152)      # row starts of the 4 AllGather splits
AG_SZ = (384, 384, 384, 128)     # rows per split
AG_BASE = (0, 3072, 6144, 9216)  # table row base of each split
AG_END_CH = (2, 5, 8, 9)         # split fires after this dense chunk

_NC = None


def ag_part(nc, agin_t, table_t, k):
    """AllGather split k of the projected rows; early splits overlap the
    tail of the producing phase. Table rows: rank-major within a split."""
    lo, sz, base = AG_LO[k], AG_SZ[k], AG_BASE[k]
    nc.gpsimd.collective_compute(
        "AllGather", mybir.AluOpType.bypass,
        replica_groups=[list(range(NCORES))],
        ins=[agin_t[lo:lo + sz, :]],
        outs=[table_t[base:base + NCORES * sz, :]])


def build():
    nc = bacc.Bacc("TRN2", num_devices=NCORES, target_bir_lowering=False)
    P = nc.declare_dram_parameter

    xT = P("xT", [F_IN, ROWS_PAD], dt.bfloat16, isOutput=False)
    w = [P(f"w{l}", [KCH[l] * 128, HID + 8], dt.bfloat16, isOutput=False)
         for l in range(L)]
    biasb = P("biasb", [128, L * HID], dt.float32, isOutput=False)
    fcw = P("fcw", [128, 8 * NCLS], dt.float32, isOutput=False)
    fcb = P("fcb", [G, NCLS], dt.float32, isOutput=False)
    crec = P("crec", [G, 1], dt.float32, isOutput=False)
    srcidx = P("srcidx", [128, NCHUNK * IDXW], dt.int16, isOutput=False)
    dstloc = P("dstloc", [128, NCHUNK * T_TILES], dt.bfloat16, isOutput=False)
    dstrow = P("dstrow", [128, NCHUNK * ESLOT], dt.bfloat16, isOutput=False)
    bo = P("bo", [128, NCHUNK * 17], dt.bfloat16, isOutput=False)
    fiota = P("fiota", [128, ESLOT], dt.bfloat16, isOutput=False)
    piota = P("piota", [128, 1], dt.float32, isOutput=False)
    out = P("out", [G, NCLS], dt.float32, isOutput=True)

    agin = [nc.dram_tensor(f"agin{l}", [ROWS_PAD, TBL], dt.int16)
            for l in range(L)]
    table = [nc.dram_tensor(f"table{l}", [TBL_ROWS, TBL], dt.int16,
                            addr_space="Shared") for l in range(L)]
    hbuf = [nc.dram_tensor(f"hbuf{l}", [ROWS_PAD, HID], dt.bfloat16)
            for l in range(L - 1)]
    arin = nc.dram_tensor("arin", [128, 136], dt.float32)
    arout = nc.dram_tensor("arout", [128, 136], dt.float32,
                           addr_space="Shared")

    with tile.TileContext(nc) as tc:
        import contextlib
        with contextlib.ExitStack() as ctx:
            const = ctx.enter_context(tc.tile_pool(name="const", bufs=1))
            wpool = ctx.enter_context(tc.tile_pool(name="wpool", bufs=1))
            hTp = ctx.enter_context(tc.tile_pool(name="hTp", bufs=1))
            gp = ctx.enter_context(tc.tile_pool(name="gp", bufs=4))
            drp = ctx.enter_context(tc.tile_pool(name="drp", bufs=3))
            s2p = ctx.enter_context(tc.tile_pool(name="s2p", bufs=3))
            stp = ctx.enter_context(tc.tile_pool(name="stp", bufs=3))
            ep = ctx.enter_context(tc.tile_pool(name="ep", bufs=3))
            hout = ctx.enter_context(tc.tile_pool(name="hout", bufs=2))
            dns = ctx.enter_context(tc.tile_pool(name="dns", bufs=2))
            edp = ctx.enter_context(tc.tile_pool(name="edp", bufs=2))
            bip = ctx.enter_context(tc.tile_pool(name="bip", bufs=1))
            psA = ctx.enter_context(tc.tile_pool(name="psA", bufs=2,
                                                 space="PSUM"))
            psX = ctx.enter_context(tc.tile_pool(name="psX", bufs=1,
                                                 space="PSUM"))
            psE = ctx.enter_context(tc.tile_pool(name="psE", bufs=1,
                                                 space="PSUM"))
            psB = ctx.enter_context(tc.tile_pool(name="psB", bufs=1,
                                                 space="PSUM"))
            psD = ctx.enter_context(tc.tile_pool(name="psD", bufs=1,
                                                 space="PSUM"))

            # ---- constants to SBUF ----
            fio = const.tile([128, ESLOT], dt.bfloat16)
            nc.sync.dma_start(fio[:], fiota[:])
            pio = const.tile([128, 1], dt.float32)
            nc.sync.dma_start(pio[:], piota[:])
            dl_sb = const.tile([128, NCHUNK * T_TILES], dt.bfloat16)
            nc.sync.dma_start(dl_sb[:], dstloc[:])
            si_sb = const.tile([128, NCHUNK * IDXW], dt.int16)
            nc.sync.dma_start(si_sb[:], srcidx[:])
            bo_sb = const.tile([128, NCHUNK * 17], dt.bfloat16)
            nc.sync.dma_start(bo_sb[:], bo[:])
            fcw_sb = const.tile([128, 8 * NCLS], dt.float32)
            nc.sync.dma_start(fcw_sb[:], fcw[:])
            fcb_sb = const.tile([G, NCLS], dt.float32)
            nc.sync.dma_start(fcb_sb[:], fcb[:])
            crec_sb = const.tile([G, 1], dt.float32)
            nc.sync.dma_start(crec_sb[:], crec[:])
            poolacc = const.tile([128, 8 * 17], dt.float32)
            nc.vector.memset(poolacc[:], 0.0)

            w_sbs = [None] * L

            def load_w(l):
                tagw = "wA" if l != 1 else "wB"
                w_sb = wpool.tile([128, 8, HID + 8], dt.bfloat16,
                                  tag=tagw, name=f"w{l}")
                for k in range(KCH[l]):
                    nc.sync.dma_start(w_sb[:, k, :],
                                      w[l][k * 128:(k + 1) * 128, :])
                w_sbs[l] = w_sb

            def dense_chunk(l, m, hT_sb, edown):
                K = KCH[l]
                w_sb = w_sbs[l]
                y1 = psD.tile([128, 512], dt.float32, space="PSUM",
                              tag="y1", name=f"y1_{l}_{m}")
                hb = dns.tile([128, HID], dt.bfloat16, tag="hb")
                for half in range(2):
                    for k in range(K):
                        nc.tensor.matmul(
                            y1[:], lhsT=hT_sb[:, k, m * 128:(m + 1) * 128],
                            rhs=w_sb[:, k, half * 512:(half + 1) * 512],
                            start=(k == 0), stop=(k == K - 1))
                    nc.vector.tensor_copy(
                        hb[:, half * 512:(half + 1) * 512], y1[:])
                for k in range(K):
                    nc.tensor.matmul(
                        y1[:, 0:8], lhsT=hT_sb[:, k, m * 128:(m + 1) * 128],
                        rhs=w_sb[:, k, 1024:1032],
                        start=(k == 0), stop=(k == K - 1))
                ee = dns.tile([128, 64], dt.float32, tag="ee")
                nc.vector.tensor_copy(ee[:, 0:8], y1[:, 0:8])
                nc.vector.memset(ee[:, 8:64], 0.0)
                nc.vector.tensor_copy(edown[:, m, :], y1[:, 4:8])
                r0 = m * 128
                nc.sync.dma_start(
                    agin[l][r0:r0 + 128, 0:HID].bitcast(dt.bfloat16), hb[:])
                nc.sync.dma_start(
                    agin[l][r0:r0 + 128, HID:TBL].bitcast(dt.float32), ee[:])

            # ---- layer 0 dense from xT ----
            load_w(0)
            xT_sb = hTp.tile([128, 8, ROWS_PAD], dt.bfloat16, tag="hT",
                             name="xT_sb")
            for k in range(KCH[0]):
                nc.sync.dma_start(xT_sb[:, k, :],
                                  xT[k * 128:(k + 1) * 128, :])
            edowns = [None] * L
            edowns[0] = edp.tile([128, NCHUNK, 4], dt.bfloat16,
                                 tag="edown", name="edown0")
            for m in range(NCHUNK):
                dense_chunk(0, m, xT_sb, edowns[0])
                if m in AG_END_CH:
                    ag_part(nc, agin[0], table[0], AG_END_CH.index(m))
            load_w(1)

            hT_next = None
            for l in range(L):
                if l < L - 1:
                    hT_next = hTp.tile([128, 8, ROWS_PAD], dt.bfloat16,
                                       tag="hT", name=f"hT{l + 1}")
                    edowns[l + 1] = edp.tile([128, NCHUNK, 4], dt.bfloat16,
                                             tag="edown",
                                             name=f"edown{l + 1}")
                if l == 1:
                    load_w(2)
                bias_l = bip.tile([128, HID], dt.float32, tag="bias",
                                  name=f"bias{l}")
                nc.sync.dma_start(bias_l[:], biasb[:, l * HID:(l + 1) * HID])
                edown = edowns[l]

                for ch in range(NCHUNK):
                    a1 = psA.tile([128, 512], dt.float32, space="PSUM",
                                  tag="x1", name=f"a1_{l}_{ch}")
                    a2 = psA.tile([128, 512], dt.float32, space="PSUM",
                                  tag="x2", name=f"a2_{l}_{ch}")
                    a3 = psX.tile([128, 8], dt.float32, space="PSUM",
                                  tag="x3", name=f"a3_{l}_{ch}")
                    dr = drp.tile([128, ESLOT], dt.bfloat16, tag="dr")
                    nc.scalar.dma_start(
                        dr[:], dstrow[:, ch * ESLOT:(ch + 1) * ESLOT])
                    # one-hot (edge -> own-dst-row), and its transpose-ish
                    # twin (own-dst-row -> edge), one op per chunk each
                    st = stp.tile([128, ESLOT], dt.bfloat16, tag="st")
                    nc.vector.tensor_tensor(
                        st[:].rearrange("p (t d) -> p t d", d=128),
                        fio[:].rearrange("p (t d) -> p t d", d=128),
                        dl_sb[:, ch * T_TILES:(ch + 1) * T_TILES]
                        .to_broadcast([128, T_TILES, 128]),
                        op=mybir.AluOpType.is_equal)
                    s2 = s2p.tile([128, ESLOT], dt.bfloat16, tag="s2")
                    nc.vector.tensor_scalar(
                        s2[:], dr[:], pio[:, 0:1], None,
                        mybir.AluOpType.is_equal)
                    for hf, (t0, T) in enumerate(HALves):
                        nidx = T * 128
                        c0 = ch * IDXW + t0 * 8
                        G_t = gp.tile([128, HT0, TBL], dt.int16, tag="G")
                        nc.gpsimd.dma_gather(
                            G_t[:, 0:T, :], table[l][:],
                            si_sb[:, c0:c0 + nidx // 16], nidx, nidx,
                            TBL, single_packet=False)
                        edx = psE.tile([128, HT0 * 4], dt.float32,
                                       space="PSUM", tag="edx",
                                       name=f"edx_{l}_{ch}_{hf}")
                        for t in range(T):
                            nc.tensor.matmul(
                                edx[:, t * 4:(t + 1) * 4],
                                lhsT=s2[:, (t0 + t) * 128:
                                        (t0 + t + 1) * 128],
                                rhs=edown[:, ch, :], start=True, stop=True)
                        es = G_t[:, 0:T, 1024:1032].bitcast(dt.float32)
                        e1 = ep.tile([128, HT0, 4], dt.float32, tag="e1")
                        nc.vector.tensor_tensor(
                            e1[:, 0:T, :], es,
                            edx[:, 0:T * 4].rearrange(
                                "p (t f) -> p t f", f=4),
                            op=mybir.AluOpType.add)
                        e2 = ep.tile([128, HT0, 4], dt.float32, tag="e2")
                        nc.vector.tensor_scalar_mul(e2[:, 0:T, :],
                                                    e1[:, 0:T, :], 0.2)
                        nc.vector.tensor_tensor(e1[:, 0:T, :],
                                                e1[:, 0:T, :],
                                                e2[:, 0:T, :],
                                                op=mybir.AluOpType.max)
                        pbf = ep.tile([128, HT0, 4], dt.bfloat16, tag="pbf")
                        nc.scalar.activation(pbf[:, 0:T, :], e1[:, 0:T, :],
                                             mybir.ActivationFunctionType.Exp)
                        # scale gathered h rows by p ((f,h)-interleaved
                        # layout keeps the innermost stream step-1 bf16)
                        for g0, gn in ((0, 5), (5, T - 5)):
                            gh = G_t[:, g0:g0 + gn, 0:HID] \
                                .bitcast(dt.bfloat16) \
                                .rearrange("p t (o h) -> p t o h", h=4)
                            pb = pbf[:, g0:g0 + gn, :] \
                                .to_broadcast([128, gn, 4, DIM]) \
                                .rearrange("p t h o -> p t o h")
                            nc.vector.tensor_tensor(
                                gh, gh, pb, op=mybir.AluOpType.mult)
                        for t in range(T):
                            tg = t0 + t
                            lhs = st[:, tg * 128:(tg + 1) * 128]
                            first = (tg == 0)
                            last = (tg == T_TILES - 1)
                            gb = G_t[:, t, :].bitcast(dt.bfloat16)
                            nc.tensor.matmul(a1[:], lhsT=lhs,
                                             rhs=gb[:, 0:512],
                                             start=first, stop=last)
                            nc.tensor.matmul(a2[:], lhsT=lhs,
                                             rhs=gb[:, 512:1024],
                                             start=first, stop=last)
                            nc.tensor.matmul(a3[:, 0:4], lhsT=lhs,
                                             rhs=pbf[:, t, :],
                                             start=first, stop=last)
                    # normalize + bias + relu
                    den = hout.tile([128, 4], dt.float32, tag="den")
                    nc.vector.tensor_scalar_max(den[:], a3[:, 0:4], 1e-30)
                    rec = hout.tile([128, 4], dt.float32, tag="rec")
                    nc.vector.reciprocal(rec[:], den[:])
                    hn = hout.tile([128, HID], dt.float32, tag="hn")
                    recb = rec[:].to_broadcast([128, 4, 128]) \
                        .rearrange("p h o -> p o h")
                    for half, ap_ in ((0, a1), (1, a2)):
                        nc.vector.tensor_tensor(
                            hn[:, half * 512:(half + 1) * 512]
                            .rearrange("p (o h) -> p o h", h=4),
                            ap_[:].rearrange("p (o h) -> p o h", h=4),
                            recb, op=mybir.AluOpType.mult)
                    nc.vector.tensor_tensor(
                        hn[:], hn[:], bias_l[:],
                        op=mybir.AluOpType.add)
                    hrelu = hout.tile([128, HID], dt.bfloat16, tag="hrelu")
                    nc.scalar.activation(hrelu[:], hn[:],
                                         mybir.ActivationFunctionType.Relu)
                    if l < L - 1:
                        nc.sync.dma_start(
                            hbuf[l][ch * 128:(ch + 1) * 128, :], hrelu[:])
                        for k in range(8):
                            nc.sync.dma_start(
                                hT_next[:, k, ch * 128:(ch + 1) * 128],
                                hbuf[l][ch * 128:(ch + 1) * 128,
                                        k * 128:(k + 1) * 128],
                                transpose=True)
                        dense_chunk(l + 1, ch, hT_next, edowns[l + 1])
                        if ch in AG_END_CH[:3]:
                            ag_part(nc, agin[l + 1], table[l + 1],
                                    AG_END_CH.index(ch))
                    else:
                        boch = bo_sb[:, ch * 17:(ch + 1) * 17]
                        pl = psB.tile([128, 8 * 17], dt.float32,
                                      space="PSUM", tag="plc",
                                      name=f"plc{ch}")
                        for kf in range(8):
                            nc.tensor.matmul(
                                pl[:, kf * 17:(kf + 1) * 17],
                                lhsT=hrelu[:, kf * 128:(kf + 1) * 128],
                                rhs=boch, start=True, stop=True)
                        nc.vector.tensor_tensor(
                            poolacc[:], poolacc[:], pl[:],
                            op=mybir.AluOpType.add)
                if l < L - 1:
                    ag_part(nc, agin[l + 1], table[l + 1], 3)

            # ---- pooling epilogue: AllReduce + FC + log_softmax ----
            nc.sync.dma_start(arin[:], poolacc[:])
            nc.gpsimd.collective_compute(
                "AllReduce", mybir.AluOpType.add,
                replica_groups=[list(range(NCORES))],
                ins=[arin[:]], outs=[arout[:]])
            pool2 = hout.tile([128, 136], dt.float32, tag="pool2")
            nc.sync.dma_start(pool2[:], arout[:])
            lgp = psB.tile([G, NCLS], dt.float32, space="PSUM",
                           tag="plc", name="lgp")
            for kf in range(8):
                nc.tensor.matmul(lgp[:], lhsT=pool2[:, kf * 17:kf * 17 + G],
                                 rhs=fcw_sb[:, kf * NCLS:(kf + 1) * NCLS],
                                 start=(kf == 0), stop=(kf == 7))
            lg = hout.tile([G, NCLS], dt.float32, tag="lg")
            nc.vector.tensor_scalar_mul(lg[:], lgp[:], crec_sb[:, 0:1])
            nc.vector.tensor_tensor(lg[:], lg[:], fcb_sb[:],
                                    op=mybir.AluOpType.add)
            mx = hout.tile([G, 1], dt.float32, tag="mx")
            nc.vector.reduce_max(mx[:], lg[:], axis=mybir.AxisListType.X)
            xs = hout.tile([G, NCLS], dt.float32, tag="xs")
            nc.vector.tensor_scalar(xs[:], lg[:], mx[:, 0:1], None,
                                    mybir.AluOpType.subtract)
            ex = hout.tile([G, NCLS], dt.float32, tag="ex")
            nc.scalar.activation(ex[:], xs[:],
                                 mybir.ActivationFunctionType.Exp)
            sm = hout.tile([G, 1], dt.float32, tag="sm")
            nc.vector.reduce_sum(sm[:], ex[:], axis=mybir.AxisListType.X)
            lnv = hout.tile([G, 1], dt.float32, tag="lnv")
            nc.scalar.activation(lnv[:], sm[:],
                                 mybir.ActivationFunctionType.Ln)
            res = hout.tile([G, NCLS], dt.float32, tag="res")
            nc.vector.tensor_scalar(res[:], xs[:], lnv[:, 0:1], None,
                                    mybir.AluOpType.subtract)
            nc.sync.dma_start(out[:], res[:])

    nc.compile()
    return nc


def _padrow(n):
    n = np.asarray(n)
    r = n // ROWS
    loc = n % ROWS
    res = np.zeros_like(n)
    for k in range(4):
        lo, sz, base = AG_LO[k], AG_SZ[k], AG_BASE[k]
        m = (loc >= lo) & (loc < lo + sz)
        res = np.where(m, base + sz * r + (loc - lo), res)
    return res


_SIG = (np.arange(HID) % HEADS) * DIM + np.arange(HID) // HEADS  # new<-old


def _wrap16(idx):
    """[n] int16 -> [128, n//16] wrapped + replicated for the 8 Q7 cores."""
    n = len(idx)
    w = np.zeros((16, n // 16), np.int16)
    w[np.arange(n) % 16, np.arange(n) // 16] = idx
    return np.tile(w, (8, 1))


def prep(x, edge_index, batch, W0, W1, W2, a_src, a_dst, bias, fc_w, fc_b):
    """Host-side sharding/prep. Returns in_maps (list of dicts per core)."""
    x = np.asarray(x, np.float32)
    edge_index = np.asarray(edge_index)
    batch = np.asarray(batch)
    Ws = [np.asarray(W0, np.float32), np.asarray(W1, np.float32),
          np.asarray(W2, np.float32)]
    a_src = np.asarray(a_src, np.float32)
    a_dst = np.asarray(a_dst, np.float32)
    bias = np.asarray(bias, np.float32)
    fc_w = np.asarray(fc_w, np.float32)
    fc_b = np.asarray(fc_b, np.float32)

    src = np.concatenate([edge_index[0], np.arange(N, dtype=np.int64)])
    dst = np.concatenate([edge_index[1], np.arange(N, dtype=np.int64)])
    order = np.argsort(dst, kind="stable")
    src, dst = src[order], dst[order]

    wext = []
    for l in range(L):
        As = np.zeros((HID, HEADS), np.float32)
        Ad = np.zeros((HID, HEADS), np.float32)
        for h in range(HEADS):
            As[h * DIM:(h + 1) * DIM, h] = a_src[l, h]
            Ad[h * DIM:(h + 1) * DIM, h] = a_dst[l, h]
        we = np.concatenate([Ws[l], Ws[l] @ As, Ws[l] @ Ad], axis=1)
        we[:, 0:HID] = we[:, _SIG]          # interleave output cols (f,h)
        if l > 0:
            we = we[_SIG, :]                # permute input rows to match
        wext.append(np.ascontiguousarray(we).astype(BF16))

    bias_p = bias[:, _SIG]
    biasb = np.broadcast_to(bias_p.reshape(1, L * HID),
                            (128, L * HID)).copy()
    fcw = fc_w[_SIG, :]
    fcw = fcw.reshape(8, 128, NCLS).transpose(1, 0, 2).reshape(128, 8 * NCLS)
    fcw = np.ascontiguousarray(fcw, np.float32)
    fcb = np.tile(fc_b.reshape(1, NCLS), (G, 1)).astype(np.float32)
    cnts = np.bincount(np.asarray(batch, np.int64),
                       minlength=G).astype(np.float32)
    crec_h = (1.0 / np.maximum(cnts, 1.0)).reshape(G, 1).astype(np.float32)
    fiota = np.broadcast_to(
        np.tile(np.arange(128, dtype=np.float32), T_TILES).astype(BF16),
        (128, ESLOT)).copy()
    piota = np.arange(128, dtype=np.float32).reshape(128, 1)

    in_maps = []
    for c in range(NCORES):
        lo, hi = c * ROWS, (c + 1) * ROWS
        m = (dst >= lo) & (dst < hi)
        s_c, d_c = src[m], dst[m] - lo
        srcidx = np.zeros((128, NCHUNK * IDXW), np.int16)
        dstloc = np.zeros((128, NCHUNK * T_TILES), BF16)
        dstrow = np.zeros((NCHUNK, ESLOT), np.float32)
        for ch in range(NCHUNK):
            mm = (d_c >= ch * 128) & (d_c < (ch + 1) * 128)
            s_e, d_e = s_c[mm], d_c[mm] - ch * 128
            cnt = len(s_e)
            assert cnt <= ESLOT, f"core {c} chunk {ch}: {cnt} > {ESLOT}"
            sfull = np.zeros(ESLOT, np.int64)
            sfull[:cnt] = _padrow(s_e)
            for t0, T in HALves:
                cc = ch * IDXW + t0 * 8
                srcidx[:, cc:cc + T * 8] = _wrap16(
                    sfull[t0 * 128:(t0 + T) * 128].astype(np.int16))
            dl = np.full(ESLOT, PAD_DST, np.float32)
            dl[:cnt] = d_e
            dstloc[:, ch * T_TILES:(ch + 1) * T_TILES] = \
                dl.reshape(T_TILES, 128).T.astype(BF16)
            dstrow[ch] = dl
        dstrow_b = np.broadcast_to(
            dstrow.reshape(1, NCHUNK * ESLOT).astype(BF16),
            (128, NCHUNK * ESLOT)).copy()
        xT = np.zeros((F_IN, ROWS_PAD), BF16)
        xT[:, :ROWS] = x[lo:hi].T.astype(BF16)
        bo = np.zeros((ROWS_PAD, 17), np.float32)
        bo[np.arange(ROWS), batch[lo:hi]] = 1.0
        bo[:ROWS, 16] = 1.0
        bo_t = np.zeros((128, NCHUNK * 17), BF16)
        for ch in range(NCHUNK):
            bo_t[:, ch * 17:(ch + 1) * 17] = \
                bo[ch * 128:(ch + 1) * 128].astype(BF16)
        in_maps.append({
            "xT": xT, "w0": wext[0], "w1": wext[1], "w2": wext[2],
            "biasb": biasb, "fcw": fcw, "fcb": fcb, "crec": crec_h,
            "srcidx": srcidx, "dstloc": dstloc, "dstrow": dstrow_b,
            "bo": bo_t, "fiota": fiota, "piota": piota,
        })
    return in_maps


def run(inputs, trace=False, **kw):
    """Returns BassKernelResults (results + exec_time_ns when trace=True)."""
    global _NC
    if _NC is None:
        _NC = build()
    in_maps = prep(**inputs)
    return run_bass_kernel_spmd(_NC, in_maps, core_ids=list(range(NCORES)),
                                trace=trace, **kw)


def kernel(**inputs) -> np.ndarray:
    r = run(inputs)
    return np.asarray(r.results[0]["out"], np.float32)
